# revision 1
# baseline (speedup 1.0000x reference)
import sys

if "/opt/trn_rl_repo" not in sys.path:
    sys.path.insert(0, "/opt/trn_rl_repo")

import numpy as np
from contextlib import ExitStack

import concourse.tile as tile
from concourse import bacc, mybir
from concourse import bass_utils

F32 = mybir.dt.float32
F32R = mybir.dt.float32r
AF = mybir.ActivationFunctionType
ALU = mybir.AluOpType
AX = mybir.AxisListType

B, C, L = 32, 128, 8192
N_CORES = 8
NB = B // N_CORES          # batches per core
CQ = C // 4
EPS = 1e-5
CH = 2048                  # P1 chunk
PCH = 1024                 # P2 chunk (2 PSUM banks)
OT = 512                   # P3 out tile (1 PSUM bank)

_BUILD_CACHE = {}


def _build(reps=1, loop_reps=0):
    key = (reps, loop_reps)
    if key in _BUILD_CACHE:
        return _BUILD_CACHE[key]

    nc = bacc.Bacc("TRN2", target_bir_lowering=False, debug=False)

    x_ap = nc.dram_tensor("x_dram", [NB, C, L], F32R, kind="ExternalInput").ap()
    w_u_ap = nc.dram_tensor("w_u", [C, C], F32, kind="ExternalInput").ap()
    wsc_aps = [nc.dram_tensor(f"wsc{k}", [C, C], F32, kind="ExternalInput").ap() for k in range(3)]
    w2t_ap = nc.dram_tensor("w2t", [C, C], F32, kind="ExternalInput").ap()
    wfc1_ap = nc.dram_tensor("wfc1", [C, CQ], F32, kind="ExternalInput").ap()
    b1e_ap = nc.dram_tensor("b1e", [CQ, 1], F32, kind="ExternalInput").ap()
    wfc2_ap = nc.dram_tensor("wfc2", [CQ, C], F32, kind="ExternalInput").ap()
    b2_ap = nc.dram_tensor("b2", [C, 1], F32, kind="ExternalInput").ap()
    t2_ap = nc.dram_tensor("t2", [C, 1], F32, kind="ExternalInput").ap()
    wam_ap = nc.dram_tensor("wam", [C, C], F32, kind="ExternalInput").ap()
    wax_ap = nc.dram_tensor("wax", [C, C], F32, kind="ExternalInput").ap()
    ident_ap = nc.dram_tensor("ident", [C, C], F32, kind="ExternalInput").ap()
    out_ap = nc.dram_tensor("out_dram", [NB, C, L], F32, kind="ExternalOutput").ap()

    with tile.TileContext(nc) as tc, ExitStack() as ctx:
        wpool = ctx.enter_context(tc.tile_pool(name="wpool", bufs=1))
        xr_pool = ctx.enter_context(tc.tile_pool(name="xr", bufs=3))
        x1_pool = ctx.enter_context(tc.tile_pool(name="x1", bufs=2))
        scr_pool = ctx.enter_context(tc.tile_pool(name="scr", bufs=2))
        m_pool = ctx.enter_context(tc.tile_pool(name="mtile", bufs=int(__import__("os").environ.get("K_MBUFS", "2"))))
        out_pool = ctx.enter_context(tc.tile_pool(name="ot", bufs=3))
        st_pool = ctx.enter_context(tc.tile_pool(name="stats", bufs=int(__import__("os").environ.get("K_STB", "2"))))
        row_pool = ctx.enter_context(tc.tile_pool(name="rows", bufs=int(__import__("os").environ.get("K_STB", "2"))))
        w2a_pool = ctx.enter_context(tc.tile_pool(name="w2a", bufs=int(__import__("os").environ.get("K_STB", "2"))))
        import os as _os
        import os as _os2
        _ub = int(_os.environ.get("K_UBUFS", "2"))
        _ob = int(_os.environ.get("K_OBUFS", "2"))
        _sb = int(_os.environ.get("K_SBUFS", "2"))
        u_psp = ctx.enter_context(tc.tile_pool(name="u_ps", bufs=_ub, space="PSUM"))
        o_psp = ctx.enter_context(tc.tile_pool(name="o_ps", bufs=_ob, space="PSUM"))
        s_psp = ctx.enter_context(tc.tile_pool(name="s_ps", bufs=_sb, space="PSUM"))
        _convfirst = _os.environ.get("K_CONVFIRST", "0") == "1"
        _outeng = _os.environ.get("K_OUTENG", "sync")
        def _dma_out(dst, srctile):
            eng = {"sync": nc.sync, "scalar": nc.scalar, "tensor": nc.tensor,
                   "gpsimd": nc.gpsimd, "vector": nc.vector}[_outeng]
            eng.dma_start(dst, srctile)

        # ---- load + prep weights (once) ----
        def wload(nm, ap, shape):
            t = wpool.tile(shape, F32, tag=nm)
            nc.sync.dma_start(t[:], ap[:])
            return t

        w_u_f = wload("w_u_f", w_u_ap, [C, C])
        wsc_f = [wload(f"wsc{k}_f", wsc_aps[k], [C, C]) for k in range(3)]
        w2t_t = wload("w2t_t", w2t_ap, [C, C])
        wfc1_t = wload("wfc1_t", wfc1_ap, [C, CQ])
        b1e_t = wload("b1e_t", b1e_ap, [CQ, 1])
        wfc2_t = wload("wfc2_t", wfc2_ap, [CQ, C])
        b2_t = wload("b2_t", b2_ap, [C, 1])
        t2_t = wload("t2_t", t2_ap, [C, 1])
        wam_t = wload("wam_t", wam_ap, [C, C])
        wax_t = wload("wax_t", wax_ap, [C, C])
        ident_t = wload("ident_t", ident_ap, [C, C])

        w_u_r = wpool.tile([C, C], F32R, tag="w_u_r")
        nc.vector.tensor_scalar(w_u_r[:], w_u_f[:], 0.0, None, ALU.add)
        wsc_r = []
        for k in range(3):
            t = wpool.tile([C, C], F32R, tag=f"wsc{k}_r")
            nc.vector.tensor_scalar(t[:], wsc_f[k][:], 0.0, None, ALU.add)
            wsc_r.append(t)
        ones_t = wpool.tile([1, C], F32, tag="ones_t")
        nc.vector.memset(ones_t[:], 1.0)

        # ---- per-batch pipeline, software-pipelined depth 2 ----
        loop_cm = tc.For_i(0, loop_reps, 1) if loop_reps else None
        if loop_cm is not None:
            loop_cm.__enter__()

        def p1_dma(b, st, ch=None):
            ch = ch or CH
            xr = xr_pool.tile([C, L + 2], F32R, tag="xr")
            st["xr"] = xr
            st["ch"] = ch
            nc.vector.memset(xr[:, 0:1].bitcast(F32), 0.0)
            nc.vector.memset(xr[:, L + 1:L + 2].bitcast(F32), 0.0)
            for q in range(L // ch):
                xsl = xr[:, 1 + q * ch:1 + (q + 1) * ch]
                nc.sync.dma_start(xsl, x_ap[b, :, q * ch:(q + 1) * ch])

        def p1_abs(b, st):
            ch = st["ch"]
            xr = st["xr"]
            sabs_p = st_pool.tile([C, 16], F32, tag="sabs_p")
            st["sabs_p"] = sabs_p
            st["sabs_n"] = L // ch
            for q in range(L // ch):
                xsl = xr[:, 1 + q * ch:1 + (q + 1) * ch]
                scr = scr_pool.tile([C, CH], F32, tag="scr")
                nc.scalar.activation(scr[:, 0:ch], xsl.bitcast(F32), AF.Abs,
                                     accum_out=sabs_p[:, q:q + 1])

        def mlp(b, st):
            sabs = st_pool.tile([C, 1], F32, tag="sabs")
            nc.vector.tensor_reduce(sabs[:], st["sabs_p"][:, 0:st["sabs_n"]], AX.X, ALU.add)
            h_ps = s_psp.tile([CQ, 1], F32, tag="s_ps")
            nc.tensor.matmul(h_ps[:], wfc1_t[:], sabs[:], start=True, stop=True)
            h_t = st_pool.tile([CQ, 1], F32, tag="h_t")
            nc.scalar.activation(h_t[:], h_ps[:], AF.Relu, bias=b1e_t[:], scale=1.0)
            y_ps = s_psp.tile([C, 1], F32, tag="s_ps")
            nc.tensor.matmul(y_ps[:], wfc2_t[:], h_t[:], start=True, stop=True)
            x12 = st_pool.tile([C, 1], F32, tag="x12")
            nc.scalar.activation(x12[:], y_ps[:], AF.Sigmoid, bias=b2_t[:], scale=1.0)
            tpos = st_pool.tile([C, 1], F32, tag="tpos")
            nc.vector.scalar_tensor_tensor(tpos[:], sabs[:], 1.0 / L, x12[:], ALU.mult, ALU.mult)
            negt = st_pool.tile([C, 1], F32, tag="negt")
            nc.vector.scalar_tensor_tensor(negt[:], sabs[:], -1.0 / L, x12[:], ALU.mult, ALU.mult)
            st["tpos"], st["negt"] = tpos, negt

        _pch = int(_os.environ.get("K_PCH", str(PCH)))

        def p2(b, st):
            xr, tpos, negt = st["xr"], st["tpos"], st["negt"]
            x1 = x1_pool.tile([C, L], F32R, tag="x1")
            st["x1"] = x1
            ssum_p = st_pool.tile([C, 16], F32, tag="ssum_p")
            st["ssum_p"] = ssum_p
            st["ssum_n"] = L // _pch
            smax_p = st_pool.tile([C, L // CH], F32, tag="smax_p")
            st["smax_p"] = smax_p
            st["smax_q"] = 0
            for p in range(L // _pch):
                u_ps = u_psp.tile([C, _pch], F32, tag="u_ps")
                base = 1 + p * _pch
                for j in range(_pch // 512):
                    nc.tensor.matmul(u_ps[:, j * 512:(j + 1) * 512], w_u_r[:],
                                     xr[:, base + j * 512:base + (j + 1) * 512],
                                     start=True, stop=True)
                m_t = m_pool.tile([C, _pch], F32, tag="m_t")
                nc.vector.scalar_tensor_tensor(m_t[:], u_ps[:], tpos[:], xr[:, base:base + _pch],
                                               ALU.add, ALU.min)
                nc.vector.scalar_tensor_tensor(x1[:, p * _pch:(p + 1) * _pch], u_ps[:], negt[:],
                                               m_t[:], ALU.add, ALU.max,
                                               accum_out=ssum_p[:, p:p + 1])
            smax_p = st_pool.tile([C, L // CH], F32, tag="smax_p")
            st["smax_p"] = smax_p
            for q in range(L // CH):
                scr2 = scr_pool.tile([C, CH], F32, tag="scr")
                nc.vector.tensor_scalar(scr2[:], x1[:, q * CH:(q + 1) * CH], 0.0, None,
                                        ALU.add, ALU.max, accum_out=smax_p[:, q:q + 1])

        def ach(b, st):
            s_x1 = st_pool.tile([C, 1], F32, tag="s_x1")
            nc.vector.tensor_reduce(s_x1[:], st["ssum_p"][:, 0:st["ssum_n"]], AX.X, ALU.add)
            mx = st_pool.tile([C, 1], F32, tag="mx")
            nc.vector.tensor_reduce(mx[:], st["smax_p"][:], AX.X, ALU.max)
            lg_ps = s_psp.tile([C, 1], F32, tag="s_ps")
            nc.tensor.matmul(lg_ps[:], wam_t[:], s_x1[:], start=True, stop=False)
            nc.tensor.matmul(lg_ps[:], wax_t[:], mx[:], start=False, stop=True)
            acol = st_pool.tile([C, 1], F32, tag="acol")
            nc.scalar.activation(acol[:], lg_ps[:], AF.Sigmoid)
            ar_ps = s_psp.tile([1, C], F32, tag="s_ps")
            nc.tensor.transpose(ar_ps[:], acol[:], ident_t[:])
            arow = row_pool.tile([1, C], F32, tag="arow")
            nc.vector.tensor_copy(arow[:], ar_ps[:])
            bc_ps = s_psp.tile([C, C], F32, tag="s_ps")
            nc.tensor.matmul(bc_ps[:], ones_t[:], arow[:], start=True, stop=True)
            w2a = w2a_pool.tile([C, C], F32R, tag="w2a")
            nc.vector.tensor_tensor(w2a[:], w2t_t[:], bc_ps[:], ALU.mult)
            st["w2a"] = w2a

        def p3(b, st, convfirst=None):
            if convfirst is None:
                convfirst = _convfirst
            xr, x1, w2a = st["xr"], st["x1"], st["w2a"]
            for i in range(L // OT):
                o_ps = o_psp.tile([C, OT], F32, tag="o_ps")
                b0 = i * OT
                if convfirst:
                    nc.tensor.matmul(o_ps[:], wsc_r[0][:], xr[:, b0:b0 + OT], start=True, stop=False)
                    nc.tensor.matmul(o_ps[:], wsc_r[1][:], xr[:, b0 + 1:b0 + 1 + OT], start=False, stop=False)
                    nc.tensor.matmul(o_ps[:], wsc_r[2][:], xr[:, b0 + 2:b0 + 2 + OT], start=False, stop=False)
                    nc.tensor.matmul(o_ps[:], w2a[:], x1[:, b0:b0 + OT], start=False, stop=True)
                else:
                    nc.tensor.matmul(o_ps[:], w2a[:], x1[:, b0:b0 + OT], start=True, stop=False)
                    nc.tensor.matmul(o_ps[:], wsc_r[0][:], xr[:, b0:b0 + OT], start=False, stop=False)
                    nc.tensor.matmul(o_ps[:], wsc_r[1][:], xr[:, b0 + 1:b0 + 1 + OT], start=False, stop=False)
                    nc.tensor.matmul(o_ps[:], wsc_r[2][:], xr[:, b0 + 2:b0 + 2 + OT], start=False, stop=True)
                ot = out_pool.tile([C, OT], F32, tag="ot")
                nc.scalar.activation(ot[:], o_ps[:], AF.Relu, bias=t2_t[:], scale=1.0)
                _dma_out(out_ap[b, :, b0:b0 + OT], ot[:])

        import os
        order = os.environ.get("K_ORDER", "a")
        seq = [b for _ in range(reps) for b in range(NB)]
        states = {}
        for s in range(len(seq) + 2):
            if order == "a":
                if s < len(seq):
                    states[s] = {}
                    p1_dma(seq[s], states[s], ch=(1024 if s == 0 else None))
                    p1_abs(seq[s], states[s])
                if 2 <= s:
                    j = s - 2
                    ach(seq[j], states[j])
                    p3(seq[j], states[j], convfirst=(j == len(seq) - 1))
                    del states[j]
                if 1 <= s <= len(seq):
                    j = s - 1
                    mlp(seq[j], states[j])
                    p2(seq[j], states[j])
            else:
                if s < len(seq):
                    states[s] = {}
                    p1_dma(seq[s], states[s])
                    p1_abs(seq[s], states[s])
                if 1 <= s <= len(seq):
                    j = s - 1
                    mlp(seq[j], states[j])
                    p2(seq[j], states[j])
                if 2 <= s:
                    j = s - 2
                    ach(seq[j], states[j])
                    p3(seq[j], states[j])
                    del states[j]

        if loop_cm is not None:
            loop_cm.__exit__(None, None, None)

    nc.compile()
    _BUILD_CACHE[key] = nc
    return nc


def _host_weights(w_fc1, b_fc1, bn1_g, bn1_b, bn1_rm, bn1_rv, w_fc2, b_fc2,
                  w1, w2, w_sp, w_sc, bn2_g, bn2_b, bn2_rm, bn2_rv):
    f = np.float32
    s1 = (bn1_g / np.sqrt(bn1_rv + EPS)).astype(f)
    t1 = (bn1_b - bn1_rm * s1).astype(f)
    wfc1 = np.ascontiguousarray(((w_fc1 * s1[:, None]) / L).T, dtype=f)      # [C, CQ]
    b1e = np.ascontiguousarray((b_fc1 * s1 + t1)[:, None], dtype=f)          # [CQ, 1]
    wfc2 = np.ascontiguousarray(w_fc2.T, dtype=f)                            # [CQ, C]
    b2 = np.ascontiguousarray(b_fc2[:, None], dtype=f)                       # [C, 1]
    w_u = np.ascontiguousarray((np.eye(C, dtype=f) + w1[:, :, 0]).T, dtype=f)
    w2t = np.ascontiguousarray(w2[:, :, 0].T, dtype=f)
    s2 = (bn2_g / np.sqrt(bn2_rv + EPS)).astype(f)
    t2 = np.ascontiguousarray((bn2_b - bn2_rm * s2)[:, None], dtype=f)
    wsc = [np.ascontiguousarray((w_sc[:, :, k] * s2[:, None]).T, dtype=f) for k in range(3)]
    # banded matrices for the channel-axis conv of [mean, max] rows:
    # logit[c] = sum_k wm_k mean[c+k-1] + sum_k wx_k max[c+k-1]  (zero-padded)
    wm = (w_sp[0, 0, :] / L).astype(f)
    wx = w_sp[0, 1, :].astype(f)
    am = (wm[0] * np.eye(C, k=-1) + wm[1] * np.eye(C) + wm[2] * np.eye(C, k=1)).astype(f)
    ax = (wx[0] * np.eye(C, k=-1) + wx[1] * np.eye(C) + wx[2] * np.eye(C, k=1)).astype(f)
    ident = np.eye(C, dtype=f)
    return {
        "w_u": w_u, "wsc0": wsc[0], "wsc1": wsc[1], "wsc2": wsc[2],
        "w2t": w2t, "wfc1": wfc1, "b1e": b1e, "wfc2": wfc2, "b2": b2,
        "t2": t2, "ident": ident,
        "wam": np.ascontiguousarray(am.T), "wax": np.ascontiguousarray(ax.T),
    }


def kernel(x, w_fc1, b_fc1, bn1_g, bn1_b, bn1_rm, bn1_rv, w_fc2, b_fc2,
           w1, w2, w_sp, w_sc, bn2_g, bn2_b, bn2_rm, bn2_rv):
    x = np.asarray(x, dtype=np.float32)
    wd = _host_weights(np.asarray(w_fc1, np.float32), np.asarray(b_fc1, np.float32),
                       np.asarray(bn1_g, np.float32), np.asarray(bn1_b, np.float32),
                       np.asarray(bn1_rm, np.float32), np.asarray(bn1_rv, np.float32),
                       np.asarray(w_fc2, np.float32), np.asarray(b_fc2, np.float32),
                       np.asarray(w1, np.float32), np.asarray(w2, np.float32),
                       np.asarray(w_sp, np.float32), np.asarray(w_sc, np.float32),
                       np.asarray(bn2_g, np.float32), np.asarray(bn2_b, np.float32),
                       np.asarray(bn2_rm, np.float32), np.asarray(bn2_rv, np.float32))

    nc = _build()
    in_maps = []
    for c in range(N_CORES):
        m = dict(wd)
        m["x_dram"] = np.ascontiguousarray(x[c * NB:(c + 1) * NB])
        in_maps.append(m)
    res = bass_utils.run_bass_kernel_spmd(nc, in_maps, core_ids=list(range(N_CORES)))
    out = np.concatenate([res.results[c]["out_dram"] for c in range(N_CORES)], axis=0)
    return out.astype(np.float32)

